# revision 39
# baseline (speedup 1.0000x reference)
"""SSIM loss kernel for Trainium2, SPMD over 8 NeuronCores.

Inputs: img1, img2 [16,3,512,512] f32. Output: scalar mean SSIM (f32).
Sharding: batch dim 16 -> 2 per core. Per-core partial pixel-sums of the
ssim map are returned as a [128] vector; host sums across partitions and
cores and divides by the element count.

Math: 11x11 Gaussian (sigma=1.5) depthwise conv, SAME padding. Uses the
u=x+y, v=x-y identity so only 4 convolutions are needed:
  m+ = conv(u), m- = conv(v), U = conv(u^2)/2, V = conv(v^2)/2
  P = m+^2/2, Q = m-^2/2
  num = (P-Q+C1) * ((U-V+C2) - (P-Q))
  den = (P+Q+C1) * ((U+V+C2) - (P+Q))
  ssim = num/den
Separable conv: pass-1 convolves H via data-stationary fp16 matmuls
(output transposed so W lands on partitions), pass-2 convolves W via
band-stationary fp16 matmuls. U+V / U-V come free from PSUM accumulation
with +/- half-scaled band copies. The fp16 Gaussian taps are tuned so
they sum to 1 at ~1e-7 (raw fp16 rounding of the taps biases sigma12 and
shifts the mean by ~9%).

Engine split: GpSimd computes u/v and the partial accumulates, Scalar
the squares (u^2, v^2, P, Q), Tensor the 4 convs, Vector the PSUM->SBUF
copies and the per-pixel rational via a fused custom-DVE op (SSIM_TERM:
(b+C1)*((a+C2)-b) in one pass), reciprocal_approx_fast, and a fused
multiply+reduce.
"""

import math

import numpy as np

from concourse import bacc, bass, mybir, tile
from concourse import dve_ops
from concourse.bass_utils import run_bass_kernel_spmd
from concourse.dve_spec import C0, C1 as DC1, Spec, Src0, Src1

B_FULL, C, H, W = 16, 3, 512, 512
N_CORES = 8
B_LOCAL = B_FULL // N_CORES          # 2
N_PLANES = B_LOCAL * C               # 6 spatial planes per core
KSZ = 11
PAD = KSZ // 2                       # 5
C1 = 0.01 ** 2
C2 = 0.03 ** 2

# 5 overlapping 128-row windows at constant stride 96; window c covers
# input rows [96c, 96c+128). Output chunk c (rows [s, s+n)) is computable
# entirely from window c (needs s-5 .. s+n+4, clipped at the image edge).
STRIDE = 96
NWIN = 5
CHUNKS = [(0, 101), (101, 96), (197, 96), (293, 96), (389, 123)]

FP32 = mybir.dt.float32
FP16 = mybir.dt.float16

# ---- fused custom-DVE op: out = (Src1 + s1) * ((Src0 + s0) - Src1) ------
# Used for num = (dP+C1)*((pD+C2)-dP) and den = (sP+C1)*((pS+C2)-sP):
# folds 3 ALU ops into one streaming DVE pass.


def _register_ssim_term():
    name = "SSIM_TERM_ANT"
    for op in dve_ops.OPS:
        if op.name == name:
            return op
    spec = Spec(
        body=(Src1 + DC1) * ((Src0 + C0) - Src1),
        reference=lambda in0, in1, s0, s1, imm2: (
            (in1.astype(np.float32) + s1) * ((in0.astype(np.float32) + s0) - in1)
        ).astype(np.float32),
    )
    op = dve_ops.DveOp(
        name,
        spec,
        subdim=False,
        uops_sha={"v3": "5c402cb9a0b28218", "v4": "71a41806c8c756d8"},
    )
    dve_ops.OPS.append(op)
    dve_ops.CUSTOM_DVE_SPECS[name] = spec
    dve_ops._SUB_OPCODE_FOR_NAME[name] = (
        dve_ops._CUSTOM_DVE_ROW_BASE + len(dve_ops.OPS) - 1
    )
    return op


SSIM_TERM = _register_ssim_term()


def _gaussian_1d_f16():
    """fp16 taps tuned so sum(g) == 1 to ~1e-7 (fix outer taps, whose fp16
    ULP is ~5e-7, to absorb the rounding of the 9 large taps)."""
    x = np.arange(KSZ)
    g = np.exp(-((x - KSZ // 2) ** 2) / (2.0 * 1.5 ** 2))
    g = g / g.sum()
    gw = g.astype(np.float16).astype(np.float64)
    mid = gw[1:KSZ - 1].sum()
    resid = (1.0 - mid) / 2.0
    gw[0] = gw[KSZ - 1] = np.float64(np.float16(resid))
    return gw


def _build_bands():
    """[128, 3*640] f16: [band | 0.5*band | -0.5*band], window c of each
    copy at cols [128c, 128c+n_c).
    out[s+jj] = sum_r band[r, 128c+jj] * x[96c + r]."""
    g = _gaussian_1d_f16()
    band = np.zeros((128, NWIN * 128), dtype=np.float64)
    for c, (s, n) in enumerate(CHUNKS):
        r0 = STRIDE * c
        for r in range(128):
            for jj in range(n):
                t = (r0 + r) - (s + jj) + PAD
                if 0 <= t < KSZ:
                    band[r, c * 128 + jj] = g[t]
    out = np.concatenate([band, 0.5 * band, -0.5 * band], axis=1)
    return out.astype(np.float16)


def _build_graph(interleave=False, copy_split=False, pipeline=True, edge_opt=True, warm=False, bufcfg=0):
    nc = bacc.Bacc()
    img1 = nc.declare_dram_parameter("img1", [B_LOCAL, C, H, W], FP32, isOutput=False)
    img2 = nc.declare_dram_parameter("img2", [B_LOCAL, C, H, W], FP32, isOutput=False)
    bands = nc.declare_dram_parameter("bands", [128, 3 * NWIN * 128], FP16, isOutput=False)
    out = nc.declare_dram_parameter("out", [128, N_PLANES * NWIN], FP32, isOutput=True)

    Alu = mybir.AluOpType
    Act = mybir.ActivationFunctionType
    INV_SQRT2 = 1.0 / math.sqrt(2.0)
    WF = NWIN * 512                  # merged free width: 5 windows x 512
    BH = NWIN * 128                  # one band copy: 640 cols

    with tile.TileContext(nc) as tc:
        with (
            tc.tile_pool(name="bands_p", bufs=1) as bands_p,
            tc.tile_pool(name="in_p", bufs=3 if bufcfg else 2) as in_p,
            tc.tile_pool(name="quv_p", bufs=2) as quv_p,
            tc.tile_pool(name="y_p", bufs=2) as y_p,
            tc.tile_pool(name="post_p", bufs=5 if bufcfg else 6) as post_p,
            tc.tile_pool(name="acc_p", bufs=1) as acc_p,
            tc.tile_pool(name="ps1_p", bufs=2, space="PSUM") as ps1_p,
            tc.tile_pool(name="psmu_p", bufs=1, space="PSUM") as psmu_p,
            tc.tile_pool(name="pssd_p", bufs=2, space="PSUM") as pssd_p,
        ):
            band_t = bands_p.tile([128, 3 * BH], FP16, name="bands")
            nc.sync.dma_start(out=band_t[:], in_=bands[:, :])

            # per-chunk partial sums land in their own column; the host does
            # the final reduction, so no on-device accumulator serializes the
            # chain across chunks/planes.
            racc_all = acc_p.tile([128, N_PLANES * NWIN], FP32, name="racc_all")

            pending = [None]
            for p in range(N_PLANES):
                b, ch = divmod(p, C)
                # --- single strided DMA per tensor: 5 overlapping windows ---
                x_m = in_p.tile([128, WF], FP32, name="x")
                y_m = in_p.tile([128, WF], FP32, name="y")
                base = (b * C + ch) * H * W
                if edge_opt and p == 0:
                    # startup: per-window DMAs + per-window u/u2 on the idle
                    # Vector/Scalar so the first pass-1 matmuls start as soon
                    # as window 0 lands (extra DMA overhead only on plane 0)
                    for dst, src_t in ((x_m, img1), (y_m, img2)):
                        for cwi in range(NWIN):
                            srcw = bass.AP(src_t, base + cwi * STRIDE * W,
                                           [[W, 128], [1, W]])
                            nc.sync.dma_start(
                                out=dst[:, cwi * 512:(cwi + 1) * 512], in_=srcw)
                else:
                    for dst, src_t in ((x_m, img1), (y_m, img2)):
                        src = bass.AP(src_t, base,
                                      [[W, 128], [STRIDE * W, NWIN], [1, W]])
                        nc.sync.dma_start(
                            out=dst[:, :].rearrange("p (c w) -> p c w", c=NWIN),
                            in_=src)

                # --- u=x+y, v=x-y (GpSimd), u2=u^2, v2=v^2 (Scalar) -------
                u_m = quv_p.tile([128, WF], FP16, name="u")
                v_m = quv_p.tile([128, WF], FP16, name="v")
                u2_m = quv_p.tile([128, WF], FP16, name="u2")
                v2_m = quv_p.tile([128, WF], FP16, name="v2")
                # plane 0: Vector is idle during startup, GpSimd is the
                # only producer -- split the first u/v across both engines
                if edge_opt and p == 0:
                    for cwi in range(NWIN):
                        sl = slice(cwi * 512, (cwi + 1) * 512)
                        nc.vector.tensor_add(u_m[:, sl], x_m[:, sl], y_m[:, sl])
                        nc.scalar.activation(u2_m[:, sl], u_m[:, sl], Act.Square)
                    nc.gpsimd.tensor_sub(v_m[:], x_m[:], y_m[:])
                    nc.scalar.activation(v2_m[:], v_m[:], Act.Square)
                else:
                    ue = nc.vector if p == 0 else nc.gpsimd
                    ue.tensor_add(u_m[:], x_m[:], y_m[:])
                    nc.gpsimd.tensor_sub(v_m[:], x_m[:], y_m[:])
                    nc.scalar.activation(u2_m[:], u_m[:], Act.Square)
                    nc.scalar.activation(v2_m[:], v_m[:], Act.Square)
                qsrc = {"u": u_m, "v": v_m, "u2": u2_m, "v2": v2_m}

                # --- pass 1: conv along H, data-stationary (output transposed)
                # col window cw covers input cols [96cw, 96cw+128) ---
                # pass-2 chunk c2 only needs column-window c2 of yv, so with
                # interleave=True it is emitted one window behind pass-1,
                # overlapping pass-2/chain with the rest of pass-1.
                yv = {q: y_p.tile([128, WF], FP16, name=f"yv_{q}")
                      for q in ("u", "v", "u2", "v2")}

                def emit_p1_group(q, cw, ei=[0]):
                    p1 = ps1_p.tile([128, 512], FP32, name="p1")
                    for c, (s, n) in enumerate(CHUNKS):
                        nc.tensor.matmul(
                            p1[:, s:s + n],
                            qsrc[q][:, c * 512 + STRIDE * cw:
                                    c * 512 + STRIDE * cw + 128],
                            band_t[:, c * 128:c * 128 + n],
                            start=True, stop=True)
                    if copy_split and ei[0] % 2 == 1:
                        nc.scalar.copy(
                            yv[q][:, cw * 512:(cw + 1) * 512], p1[:, :])
                    else:
                        nc.vector.tensor_copy(
                            yv[q][:, cw * 512:(cw + 1) * 512], p1[:, :])
                    ei[0] += 1

                # --- pass 2: conv along W (band stationary) + post math ---
                # full-128 band slices: rows beyond n2 produce zeros (the
                # band cols there are zero), so chain ops run on [128, 512]
                # uniformly; only the final accumulate is sliced to n2.
                def emit_p2_chunk(c2, pp=None, yvv=None):
                    pp = p if pp is None else pp
                    yvv = yv if yvv is None else yvv
                    s2, n2 = CHUNKS[c2]
                    pm_u = psmu_p.tile([128, 512], FP32, name="pm_u")
                    pm_v = psmu_p.tile([128, 512], FP32, name="pm_v")
                    pS = pssd_p.tile([128, 512], FP32, name="pS")
                    pD = pssd_p.tile([128, 512], FP32, name="pD")
                    bsl = slice(c2 * 128, (c2 + 1) * 128)
                    ysl = slice(c2 * 512, (c2 + 1) * 512)
                    nc.tensor.matmul(
                        pm_u[:, :], band_t[:, bsl], yvv["u"][:, ysl],
                        start=True, stop=True)
                    nc.tensor.matmul(
                        pm_v[:, :], band_t[:, bsl], yvv["v"][:, ysl],
                        start=True, stop=True)
                    bh = slice(BH + c2 * 128, BH + (c2 + 1) * 128)
                    bnh = slice(2 * BH + c2 * 128, 2 * BH + (c2 + 1) * 128)
                    nc.tensor.matmul(
                        pS[:, :], band_t[:, bh], yvv["u2"][:, ysl],
                        start=True, stop=False)
                    nc.tensor.matmul(
                        pS[:, :], band_t[:, bh], yvv["v2"][:, ysl],
                        start=False, stop=True)
                    nc.tensor.matmul(
                        pD[:, :], band_t[:, bh], yvv["u2"][:, ysl],
                        start=True, stop=False)
                    nc.tensor.matmul(
                        pD[:, :], band_t[:, bnh], yvv["v2"][:, ysl],
                        start=False, stop=True)

                    # P = m+^2/2, Q = m-^2/2 (Scalar, PSUM-adjacent)
                    P = post_p.tile([128, 512], FP16, name="P")
                    Q = post_p.tile([128, 512], FP16, name="Q")
                    nc.scalar.activation(P[:], pm_u[:], Act.Square, scale=INV_SQRT2)
                    nc.scalar.activation(Q[:], pm_v[:], Act.Square, scale=INV_SQRT2)
                    dP = post_p.tile([128, 512], FP16, name="dP")
                    sP = post_p.tile([128, 512], FP16, name="sP")
                    nc.vector.tensor_sub(dP[:], P[:], Q[:])
                    nc.vector.tensor_add(sP[:], P[:], Q[:])
                    num = post_p.tile([128, 512], FP16, name="num")
                    den = post_p.tile([128, 512], FP32, name="den")
                    nc.vector._custom_dve(
                        SSIM_TERM, out=num[:], in0=pD[:], in1=dP[:],
                        s0=C2, s1=C1)
                    nc.vector._custom_dve(
                        SSIM_TERM, out=den[:], in0=pS[:], in1=sP[:],
                        s0=C2, s1=C1)
                    rec = post_p.tile([128, 512], FP32, name="rec")
                    nc.vector.reciprocal_approx_fast(rec[:], den[:])
                    scr = post_p.tile([128, 512], FP32, name="scr")
                    idx = pp * NWIN + c2
                    nc.vector.scalar_tensor_tensor(
                        scr[:], num[:], 0.0, rec[:], Alu.add, Alu.mult,
                        accum_out=racc_all[:, idx:idx + 1])
                    if warm and c2 < NWIN - 1:
                        # dependency-timed PE pokes: standalone fp16 LDWs that
                        # wait on chain tiles fire mid-drain, keeping the HAM
                        # activity monitor from dropping the PE to 1.2 GHz
                        # during the chain phase (weights are overwritten by
                        # the next real matmul's own load).
                        nc.tensor.ldweights(dP[:, :128])
                        nc.tensor.ldweights(sP[:, :128])
                        nc.tensor.ldweights(num[:, :128])

                if interleave:
                    for cw in range(NWIN):
                        for q in ("u", "v", "u2", "v2"):
                            emit_p1_group(q, cw)
                        if cw >= 1:
                            emit_p2_chunk(cw - 1)
                    emit_p2_chunk(NWIN - 1)
                elif pipeline:
                    # coarse cross-plane pipeline: pass-1(p) first (its copies
                    # precede chain(p-1) in Vector's stream, so ps1 banks free
                    # promptly), then pass-2+chain of the previous plane.
                    if edge_opt and p == N_PLANES - 1:
                        # last plane: no next plane can be blocked, so
                        # interleave its own pass-2 (and the pending one)
                        # into pass-1 to shrink the drain tail
                        for cw in range(NWIN):
                            for q in ("u", "v", "u2", "v2"):
                                emit_p1_group(q, cw)
                            if pending[0] is not None:
                                pending[0][cw]()
                            if cw >= 1:
                                emit_p2_chunk(cw - 1)
                        emit_p2_chunk(NWIN - 1)
                        pending[0] = None
                    else:
                        for q in ("u", "v", "u2", "v2"):
                            for cw in range(NWIN):
                                emit_p1_group(q, cw)
                        if pending[0] is not None:
                            for c2 in range(NWIN):
                                pending[0][c2]()
                        pending[0] = [
                            (lambda cc=c2, f=emit_p2_chunk, pb=p, yb=yv:
                             f(cc, pb, yb)) for c2 in range(NWIN)]
                else:
                    for q in ("u", "v", "u2", "v2"):
                        for cw in range(NWIN):
                            emit_p1_group(q, cw)
                    for c2 in range(NWIN):
                        emit_p2_chunk(c2)

            if pipeline and pending[0] is not None:
                for c2 in range(NWIN):
                    pending[0][c2]()

            nc.sync.dma_start(out=out[:, :], in_=racc_all[:])

    nc.compile()
    return nc


_NC_CACHE = None


def kernel(img1: np.ndarray, img2: np.ndarray) -> np.ndarray:
    global _NC_CACHE
    if _NC_CACHE is None:
        _NC_CACHE = _build_graph()
    nc = _NC_CACHE

    img1 = np.ascontiguousarray(img1, dtype=np.float32)
    img2 = np.ascontiguousarray(img2, dtype=np.float32)
    bands = _build_bands()
    in_maps = [
        {
            "img1": img1[i * B_LOCAL:(i + 1) * B_LOCAL],
            "img2": img2[i * B_LOCAL:(i + 1) * B_LOCAL],
            "bands": bands,
        }
        for i in range(N_CORES)
    ]
    res = run_bass_kernel_spmd(nc, in_maps, list(range(N_CORES)))
    total = np.float64(0.0)
    for r in res.results:
        racc = np.asarray(r["out"], dtype=np.float64)  # [128, planes*chunks]
        for idx in range(N_PLANES * NWIN):
            n2 = CHUNKS[idx % NWIN][1]
            total += racc[:n2, idx].sum()
    mean = total / (B_FULL * C * H * W)
    return np.array(mean, dtype=np.float32)


# revision 40
# speedup vs baseline: 1.0054x; 1.0054x over previous
"""SSIM loss kernel for Trainium2, SPMD over 8 NeuronCores.

Inputs: img1, img2 [16,3,512,512] f32. Output: scalar mean SSIM (f32).
Sharding: batch dim 16 -> 2 per core. Per-core partial pixel-sums of the
ssim map are returned as a [128] vector; host sums across partitions and
cores and divides by the element count.

Math: 11x11 Gaussian (sigma=1.5) depthwise conv, SAME padding. Uses the
u=x+y, v=x-y identity so only 4 convolutions are needed:
  m+ = conv(u), m- = conv(v), U = conv(u^2)/2, V = conv(v^2)/2
  P = m+^2/2, Q = m-^2/2
  num = (P-Q+C1) * ((U-V+C2) - (P-Q))
  den = (P+Q+C1) * ((U+V+C2) - (P+Q))
  ssim = num/den
Separable conv: pass-1 convolves H via data-stationary fp16 matmuls
(output transposed so W lands on partitions), pass-2 convolves W via
band-stationary fp16 matmuls. U+V / U-V come free from PSUM accumulation
with +/- half-scaled band copies. The fp16 Gaussian taps are tuned so
they sum to 1 at ~1e-7 (raw fp16 rounding of the taps biases sigma12 and
shifts the mean by ~9%).

Engine split: GpSimd computes u/v and the partial accumulates, Scalar
the squares (u^2, v^2, P, Q), Tensor the 4 convs, Vector the PSUM->SBUF
copies and the per-pixel rational via a fused custom-DVE op (SSIM_TERM:
(b+C1)*((a+C2)-b) in one pass), reciprocal_approx_fast, and a fused
multiply+reduce.
"""

import math

import numpy as np

from concourse import bacc, bass, mybir, tile
from concourse import dve_ops
from concourse.bass_utils import run_bass_kernel_spmd
from concourse.dve_spec import C0, C1 as DC1, Spec, Src0, Src1

B_FULL, C, H, W = 16, 3, 512, 512
N_CORES = 8
B_LOCAL = B_FULL // N_CORES          # 2
N_PLANES = B_LOCAL * C               # 6 spatial planes per core
KSZ = 11
PAD = KSZ // 2                       # 5
C1 = 0.01 ** 2
C2 = 0.03 ** 2

# 5 overlapping 128-row windows at constant stride 96; window c covers
# input rows [96c, 96c+128). Output chunk c (rows [s, s+n)) is computable
# entirely from window c (needs s-5 .. s+n+4, clipped at the image edge).
STRIDE = 96
NWIN = 5
CHUNKS = [(0, 101), (101, 96), (197, 96), (293, 96), (389, 123)]

FP32 = mybir.dt.float32
FP16 = mybir.dt.float16

# ---- fused custom-DVE op: out = (Src1 + s1) * ((Src0 + s0) - Src1) ------
# Used for num = (dP+C1)*((pD+C2)-dP) and den = (sP+C1)*((pS+C2)-sP):
# folds 3 ALU ops into one streaming DVE pass.


def _register_ssim_term():
    name = "SSIM_TERM_ANT"
    for op in dve_ops.OPS:
        if op.name == name:
            return op
    spec = Spec(
        body=(Src1 + DC1) * ((Src0 + C0) - Src1),
        reference=lambda in0, in1, s0, s1, imm2: (
            (in1.astype(np.float32) + s1) * ((in0.astype(np.float32) + s0) - in1)
        ).astype(np.float32),
    )
    op = dve_ops.DveOp(
        name,
        spec,
        subdim=False,
        uops_sha={"v3": "5c402cb9a0b28218", "v4": "71a41806c8c756d8"},
    )
    dve_ops.OPS.append(op)
    dve_ops.CUSTOM_DVE_SPECS[name] = spec
    dve_ops._SUB_OPCODE_FOR_NAME[name] = (
        dve_ops._CUSTOM_DVE_ROW_BASE + len(dve_ops.OPS) - 1
    )
    return op


SSIM_TERM = _register_ssim_term()


def _register_pair_subadd():
    name = "PAIR_SUBADD_ANT"
    for op in dve_ops.OPS:
        if op.name == name:
            return op
    from concourse.dve_spec import Idx, select, lower as _lower
    from concourse.dve_uop import DveOpSpec as _DOS
    spec = Spec(
        body=select(Idx < C0, Src0 - Src1, Src0 + Src1),
        reference=lambda in0, in1, s0, s1, imm2: in0.astype(np.float32),
    )
    row = dve_ops._CUSTOM_DVE_ROW_BASE + len(dve_ops.OPS)
    shas = {}
    for ver in ("v3", "v4"):
        u = _lower(spec, ver=ver)
        shas[ver] = _DOS(name=name, opcode=row, uops=u, rd1_en=True).sha(ver)
    op = dve_ops.DveOp(name, spec, subdim=False, uops_sha=shas)
    dve_ops.OPS.append(op)
    dve_ops.CUSTOM_DVE_SPECS[name] = spec
    dve_ops._SUB_OPCODE_FOR_NAME[name] = row
    return op


PAIR_SUBADD = _register_pair_subadd()


def _gaussian_1d_f16():
    """fp16 taps tuned so sum(g) == 1 to ~1e-7 (fix outer taps, whose fp16
    ULP is ~5e-7, to absorb the rounding of the 9 large taps)."""
    x = np.arange(KSZ)
    g = np.exp(-((x - KSZ // 2) ** 2) / (2.0 * 1.5 ** 2))
    g = g / g.sum()
    gw = g.astype(np.float16).astype(np.float64)
    mid = gw[1:KSZ - 1].sum()
    resid = (1.0 - mid) / 2.0
    gw[0] = gw[KSZ - 1] = np.float64(np.float16(resid))
    return gw


def _build_bands():
    """[128, 3*640] f16: [band | 0.5*band | -0.5*band], window c of each
    copy at cols [128c, 128c+n_c).
    out[s+jj] = sum_r band[r, 128c+jj] * x[96c + r]."""
    g = _gaussian_1d_f16()
    band = np.zeros((128, NWIN * 128), dtype=np.float64)
    for c, (s, n) in enumerate(CHUNKS):
        r0 = STRIDE * c
        for r in range(128):
            for jj in range(n):
                t = (r0 + r) - (s + jj) + PAD
                if 0 <= t < KSZ:
                    band[r, c * 128 + jj] = g[t]
    out = np.concatenate([band, 0.5 * band, -0.5 * band], axis=1)
    return out.astype(np.float16)


def _build_graph(interleave=False, copy_split=False, pipeline=True, edge_opt=True, warm=False, bufcfg=0, pair_ds=False):
    nc = bacc.Bacc()
    img1 = nc.declare_dram_parameter("img1", [B_LOCAL, C, H, W], FP32, isOutput=False)
    img2 = nc.declare_dram_parameter("img2", [B_LOCAL, C, H, W], FP32, isOutput=False)
    bands = nc.declare_dram_parameter("bands", [128, 3 * NWIN * 128], FP16, isOutput=False)
    out = nc.declare_dram_parameter("out", [128, N_PLANES * NWIN], FP32, isOutput=True)

    Alu = mybir.AluOpType
    Act = mybir.ActivationFunctionType
    INV_SQRT2 = 1.0 / math.sqrt(2.0)
    WF = NWIN * 512                  # merged free width: 5 windows x 512
    BH = NWIN * 128                  # one band copy: 640 cols

    with tile.TileContext(nc) as tc:
        with (
            tc.tile_pool(name="bands_p", bufs=1) as bands_p,
            tc.tile_pool(name="in_p", bufs=3 if bufcfg else 2) as in_p,
            tc.tile_pool(name="quv_p", bufs=2) as quv_p,
            tc.tile_pool(name="y_p", bufs=2) as y_p,
            tc.tile_pool(name="post_p", bufs=5 if bufcfg else 6) as post_p,
            tc.tile_pool(name="acc_p", bufs=1) as acc_p,
            tc.tile_pool(name="ps1_p", bufs=2, space="PSUM") as ps1_p,
            tc.tile_pool(name="psmu_p", bufs=1, space="PSUM") as psmu_p,
            tc.tile_pool(name="pssd_p", bufs=2, space="PSUM") as pssd_p,
        ):
            band_t = bands_p.tile([128, 3 * BH], FP16, name="bands")
            nc.sync.dma_start(out=band_t[:], in_=bands[:, :])

            # per-chunk partial sums land in their own column; the host does
            # the final reduction, so no on-device accumulator serializes the
            # chain across chunks/planes.
            racc_all = acc_p.tile([128, N_PLANES * NWIN], FP32, name="racc_all")

            pending = [None]
            for p in range(N_PLANES):
                b, ch = divmod(p, C)
                # --- single strided DMA per tensor: 5 overlapping windows ---
                x_m = in_p.tile([128, WF], FP32, name="x")
                y_m = in_p.tile([128, WF], FP32, name="y")
                base = (b * C + ch) * H * W
                if edge_opt and p == 0:
                    # startup: per-window DMAs + per-window u/u2 on the idle
                    # Vector/Scalar so the first pass-1 matmuls start as soon
                    # as window 0 lands (extra DMA overhead only on plane 0)
                    for dst, src_t in ((x_m, img1), (y_m, img2)):
                        for cwi in range(NWIN):
                            srcw = bass.AP(src_t, base + cwi * STRIDE * W,
                                           [[W, 128], [1, W]])
                            nc.sync.dma_start(
                                out=dst[:, cwi * 512:(cwi + 1) * 512], in_=srcw)
                else:
                    for dst, src_t in ((x_m, img1), (y_m, img2)):
                        src = bass.AP(src_t, base,
                                      [[W, 128], [STRIDE * W, NWIN], [1, W]])
                        nc.sync.dma_start(
                            out=dst[:, :].rearrange("p (c w) -> p c w", c=NWIN),
                            in_=src)

                # --- u=x+y, v=x-y (GpSimd), u2=u^2, v2=v^2 (Scalar) -------
                u_m = quv_p.tile([128, WF], FP16, name="u")
                v_m = quv_p.tile([128, WF], FP16, name="v")
                u2_m = quv_p.tile([128, WF], FP16, name="u2")
                v2_m = quv_p.tile([128, WF], FP16, name="v2")
                # plane 0: Vector is idle during startup, GpSimd is the
                # only producer -- split the first u/v across both engines
                if edge_opt and p == 0:
                    for cwi in range(NWIN):
                        sl = slice(cwi * 512, (cwi + 1) * 512)
                        nc.vector.tensor_add(u_m[:, sl], x_m[:, sl], y_m[:, sl])
                        nc.scalar.activation(u2_m[:, sl], u_m[:, sl], Act.Square)
                    nc.gpsimd.tensor_sub(v_m[:], x_m[:], y_m[:])
                    nc.scalar.activation(v2_m[:], v_m[:], Act.Square)
                else:
                    ue = nc.vector if p == 0 else nc.gpsimd
                    ue.tensor_add(u_m[:], x_m[:], y_m[:])
                    nc.gpsimd.tensor_sub(v_m[:], x_m[:], y_m[:])
                    nc.scalar.activation(u2_m[:], u_m[:], Act.Square)
                    nc.scalar.activation(v2_m[:], v_m[:], Act.Square)
                qsrc = {"u": u_m, "v": v_m, "u2": u2_m, "v2": v2_m}

                # --- pass 1: conv along H, data-stationary (output transposed)
                # col window cw covers input cols [96cw, 96cw+128) ---
                # pass-2 chunk c2 only needs column-window c2 of yv, so with
                # interleave=True it is emitted one window behind pass-1,
                # overlapping pass-2/chain with the rest of pass-1.
                yv = {q: y_p.tile([128, WF], FP16, name=f"yv_{q}")
                      for q in ("u", "v", "u2", "v2")}

                def emit_p1_group(q, cw, ei=[0]):
                    p1 = ps1_p.tile([128, 512], FP32, name="p1")
                    for c, (s, n) in enumerate(CHUNKS):
                        nc.tensor.matmul(
                            p1[:, s:s + n],
                            qsrc[q][:, c * 512 + STRIDE * cw:
                                    c * 512 + STRIDE * cw + 128],
                            band_t[:, c * 128:c * 128 + n],
                            start=True, stop=True)
                    if copy_split and ei[0] % 2 == 1:
                        nc.scalar.copy(
                            yv[q][:, cw * 512:(cw + 1) * 512], p1[:, :])
                    else:
                        nc.vector.tensor_copy(
                            yv[q][:, cw * 512:(cw + 1) * 512], p1[:, :])
                    ei[0] += 1

                # --- pass 2: conv along W (band stationary) + post math ---
                # full-128 band slices: rows beyond n2 produce zeros (the
                # band cols there are zero), so chain ops run on [128, 512]
                # uniformly; only the final accumulate is sliced to n2.
                def emit_p2_chunk(c2, pp=None, yvv=None):
                    pp = p if pp is None else pp
                    yvv = yv if yvv is None else yvv
                    s2, n2 = CHUNKS[c2]
                    pm_u = psmu_p.tile([128, 512], FP32, name="pm_u")
                    pm_v = psmu_p.tile([128, 512], FP32, name="pm_v")
                    pS = pssd_p.tile([128, 512], FP32, name="pS")
                    pD = pssd_p.tile([128, 512], FP32, name="pD")
                    bsl = slice(c2 * 128, (c2 + 1) * 128)
                    ysl = slice(c2 * 512, (c2 + 1) * 512)
                    nc.tensor.matmul(
                        pm_u[:, :], band_t[:, bsl], yvv["u"][:, ysl],
                        start=True, stop=True)
                    nc.tensor.matmul(
                        pm_v[:, :], band_t[:, bsl], yvv["v"][:, ysl],
                        start=True, stop=True)
                    bh = slice(BH + c2 * 128, BH + (c2 + 1) * 128)
                    bnh = slice(2 * BH + c2 * 128, 2 * BH + (c2 + 1) * 128)
                    nc.tensor.matmul(
                        pS[:, :], band_t[:, bh], yvv["u2"][:, ysl],
                        start=True, stop=False)
                    nc.tensor.matmul(
                        pS[:, :], band_t[:, bh], yvv["v2"][:, ysl],
                        start=False, stop=True)
                    nc.tensor.matmul(
                        pD[:, :], band_t[:, bh], yvv["u2"][:, ysl],
                        start=True, stop=False)
                    nc.tensor.matmul(
                        pD[:, :], band_t[:, bnh], yvv["v2"][:, ysl],
                        start=False, stop=True)

                    # P = m+^2/2, Q = m-^2/2 (Scalar, PSUM-adjacent)
                    P = post_p.tile([128, 512], FP16, name="P")
                    Q = post_p.tile([128, 512], FP16, name="Q")
                    nc.scalar.activation(P[:], pm_u[:], Act.Square, scale=INV_SQRT2)
                    nc.scalar.activation(Q[:], pm_v[:], Act.Square, scale=INV_SQRT2)
                    if pair_ds:
                        dPsP = post_p.tile([128, 1024], FP16, name="dPsP")
                        Pb = P[:, :].rearrange("p (o w) -> p o w", o=1)\
                            .broadcast_to((128, 2, 512))
                        Qb = Q[:, :].rearrange("p (o w) -> p o w", o=1)\
                            .broadcast_to((128, 2, 512))
                        nc.vector._custom_dve(
                            PAIR_SUBADD,
                            out=dPsP[:, :].rearrange("p (o w) -> p o w", o=2),
                            in0=Pb, in1=Qb, s0=512.0)
                        dP_ap, sP_ap = dPsP[:, :512], dPsP[:, 512:]
                    else:
                        dP = post_p.tile([128, 512], FP16, name="dP")
                        sP = post_p.tile([128, 512], FP16, name="sP")
                        nc.vector.tensor_sub(dP[:], P[:], Q[:])
                        nc.vector.tensor_add(sP[:], P[:], Q[:])
                        dP_ap, sP_ap = dP[:], sP[:]
                    num = post_p.tile([128, 512], FP16, name="num")
                    den = post_p.tile([128, 512], FP32, name="den")
                    nc.vector._custom_dve(
                        SSIM_TERM, out=num[:], in0=pD[:], in1=dP_ap,
                        s0=C2, s1=C1)
                    nc.vector._custom_dve(
                        SSIM_TERM, out=den[:], in0=pS[:], in1=sP_ap,
                        s0=C2, s1=C1)
                    rec = post_p.tile([128, 512], FP32, name="rec")
                    nc.vector.reciprocal_approx_fast(rec[:], den[:])
                    scr = post_p.tile([128, 512], FP32, name="scr")
                    idx = pp * NWIN + c2
                    nc.vector.scalar_tensor_tensor(
                        scr[:], num[:], 0.0, rec[:], Alu.add, Alu.mult,
                        accum_out=racc_all[:, idx:idx + 1])
                    if warm and c2 < NWIN - 1:
                        # dependency-timed PE pokes: standalone fp16 LDWs that
                        # wait on chain tiles fire mid-drain, keeping the HAM
                        # activity monitor from dropping the PE to 1.2 GHz
                        # during the chain phase (weights are overwritten by
                        # the next real matmul's own load).
                        nc.tensor.ldweights(dP[:, :128])
                        nc.tensor.ldweights(sP[:, :128])
                        nc.tensor.ldweights(num[:, :128])

                if interleave:
                    for cw in range(NWIN):
                        for q in ("u", "v", "u2", "v2"):
                            emit_p1_group(q, cw)
                        if cw >= 1:
                            emit_p2_chunk(cw - 1)
                    emit_p2_chunk(NWIN - 1)
                elif pipeline:
                    # coarse cross-plane pipeline: pass-1(p) first (its copies
                    # precede chain(p-1) in Vector's stream, so ps1 banks free
                    # promptly), then pass-2+chain of the previous plane.
                    if edge_opt and p == N_PLANES - 1:
                        # last plane: no next plane can be blocked, so
                        # interleave its own pass-2 (and the pending one)
                        # into pass-1 to shrink the drain tail
                        for cw in range(NWIN):
                            for q in ("u", "v", "u2", "v2"):
                                emit_p1_group(q, cw)
                            if pending[0] is not None:
                                pending[0][cw]()
                            if cw >= 1:
                                emit_p2_chunk(cw - 1)
                        emit_p2_chunk(NWIN - 1)
                        pending[0] = None
                    else:
                        for q in ("u", "v", "u2", "v2"):
                            for cw in range(NWIN):
                                emit_p1_group(q, cw)
                        if pending[0] is not None:
                            for c2 in range(NWIN):
                                pending[0][c2]()
                        pending[0] = [
                            (lambda cc=c2, f=emit_p2_chunk, pb=p, yb=yv:
                             f(cc, pb, yb)) for c2 in range(NWIN)]
                else:
                    for q in ("u", "v", "u2", "v2"):
                        for cw in range(NWIN):
                            emit_p1_group(q, cw)
                    for c2 in range(NWIN):
                        emit_p2_chunk(c2)

            if pipeline and pending[0] is not None:
                for c2 in range(NWIN):
                    pending[0][c2]()

            nc.sync.dma_start(out=out[:, :], in_=racc_all[:])

    nc.compile()
    return nc


_NC_CACHE = None


def kernel(img1: np.ndarray, img2: np.ndarray) -> np.ndarray:
    global _NC_CACHE
    if _NC_CACHE is None:
        _NC_CACHE = _build_graph()
    nc = _NC_CACHE

    img1 = np.ascontiguousarray(img1, dtype=np.float32)
    img2 = np.ascontiguousarray(img2, dtype=np.float32)
    bands = _build_bands()
    in_maps = [
        {
            "img1": img1[i * B_LOCAL:(i + 1) * B_LOCAL],
            "img2": img2[i * B_LOCAL:(i + 1) * B_LOCAL],
            "bands": bands,
        }
        for i in range(N_CORES)
    ]
    res = run_bass_kernel_spmd(nc, in_maps, list(range(N_CORES)))
    total = np.float64(0.0)
    for r in res.results:
        racc = np.asarray(r["out"], dtype=np.float64)  # [128, planes*chunks]
        for idx in range(N_PLANES * NWIN):
            n2 = CHUNKS[idx % NWIN][1]
            total += racc[:n2, idx].sum()
    mean = total / (B_FULL * C * H * W)
    return np.array(mean, dtype=np.float32)
